# revision 5
# baseline (speedup 1.0000x reference)
"""MGCN (2-layer relational GCN, basis decomposition + segment softmax) on
8 Trainium2 NeuronCores via Bass/Tile, SPMD.

Sharding: nodes are range-partitioned across the 8 cores (2500 each); every
core owns the edges whose *destination* lands in its range, so the segment
softmax and the scatter-add are core-local.  Node features and parameters are
replicated.  One bf16 AllGather of the hidden layer between the two convs is
the only collective.

Per layer, on device (all SPMD-static; per-core data arrives as index
tensors):
  1. transpose-gathers of x_j^T / x_i^T / w[et]^T (bf16, [i, e] layout)
  2. attention logits alpha = sum_i x_i*w*x_j   (DVE muls + ones-matmul)
  3. a = exp(alpha)  (segment-max skipped: |alpha| <~ 6, exp is safe)
  4. W_r = sum_b att[r, b] basis[b]  via per-o matmuls into [i, (o, r)]
  5. relation-grouped GEMM msgT[o, e] = W_et^T x_j^T  (edges in a cross-core
     relation template so the matmul sequence is identical on all cores)
  6. HBM bounce: msgT -> msg rows -> permutation gather into dst-sorted,
     tile-aligned segment order
  7. per-tile a-scaled one-hot segment matmuls -> softmax numerator + denom,
     normalize with reciprocal
  8. dma_scatter_add (unique indices by construction) into the per-core
     output accumulator; root term + bias added on read-back
"""

import os
import sys

import numpy as np

sys.path.insert(0, "/opt/trn_rl_repo")

import ml_dtypes

N_NODES = 20000
D = 128
R = 474
B = 64
NCORES = 8
NPC = N_NODES // NCORES            # 2500 nodes per core
OWN_PAD = 2560                     # NPC padded to tile multiple
TRASH = NPC                        # scatter target for padding/empty slots

bf16 = ml_dtypes.bfloat16


# ----------------------------------------------------------------------------
# host-side schedule construction (pure numpy, data-dependent, SPMD-uniform)
# ----------------------------------------------------------------------------

def _wrap_idx(idx):
    """int array [n] (n % 16 == 0) -> [128, n/16] int16 wrapped/replicated."""
    idx = np.asarray(idx, dtype=np.int16)
    n = idx.shape[0]
    blk = idx.reshape(n // 16, 16).T
    return np.ascontiguousarray(np.tile(blk, (8, 1)))


def _build_schedule(src, dst, et):
    E = src.shape[0]
    core_of = dst // NPC

    # ---- relation template: slot range per relation, shared by all cores ----
    # counts[c, r]
    counts = np.zeros((NCORES, R), dtype=np.int64)
    np.add.at(counts, (core_of, et), 1)
    L = counts.max(axis=0)                      # [R] slots per relation
    active = np.nonzero(L > 0)[0]
    S = np.zeros(R + 1, dtype=np.int64)
    S[1:] = np.cumsum(L)
    EP = int(-(-S[R] // 128) * 128)             # padded template length
    # inflate the last active relation so the template covers [0, EP) exactly
    # (every psum column gets written; pad columns compute real-but-unused
    # messages for gather row 0)
    L[active[-1]] += EP - S[R]
    S[1:] = np.cumsum(L)

    # GEMM segment list: (relation, lo, hi) split at 512-edge psum chunks and
    # at the W chunk boundary (relations 0..RC-1 / RC..R-1 in template order).
    RC = 237                                    # relations per W chunk
    segs = []
    for r in active:
        lo, hi = int(S[r]), int(S[r] + L[r])
        while lo < hi:
            nxt = min(hi, (lo // 512 + 1) * 512)
            segs.append((int(r), lo, nxt))
            lo = nxt

    per_core = []
    dst_lists = []
    for c in range(NCORES):
        mask = core_of == c
        eids = np.nonzero(mask)[0]
        # type-template slots
        gsrc_t = np.zeros(EP, dtype=np.int64)
        slot_of_edge = np.empty(eids.shape[0], dtype=np.int64)
        order = np.argsort(et[eids], kind="stable")
        eids_sorted = eids[order]
        ets = et[eids_sorted]
        # place edges of relation r at S[r]..
        start = np.searchsorted(ets, np.arange(R))
        for r in active:
            n_rc = int(counts[c, r])
            sl = int(S[r])
            gsrc_t[sl:sl + n_rc] = src[eids_sorted[start[r]:start[r] + n_rc]]
            slot_of_edge[start[r]:start[r] + n_rc] = sl + np.arange(n_rc)
        per_core.append(dict(gsrc_t=gsrc_t))
        # dst-sorted edge walk (local node ids), segments never cross tiles
        dloc = dst[eids_sorted] - c * NPC
        dorder = np.argsort(dloc, kind="stable")
        dst_lists.append((eids_sorted, slot_of_edge, dloc, dorder))

    # ---- dst-sorted tiling with per-tile segment slots -----------------------
    # first pass per core to learn tile counts, then pad all cores to EPL
    walks = []
    ntiles_max = 0
    for c in range(NCORES):
        eids_sorted, slot_of_edge, dloc, dorder = dst_lists[c]
        d_sorted = dloc[dorder]
        tmpl_pos = slot_of_edge[dorder]          # template row of each edge
        # group boundaries
        uniq, ustart = np.unique(d_sorted, return_index=True)
        uend = np.append(ustart[1:], d_sorted.shape[0])
        tiles = []                               # list of (rows, segids, segdst)
        cur_rows, cur_seg, cur_dst = [], [], []
        nseg = 0
        for u, s0, e0 in zip(uniq, ustart, uend):
            glen = e0 - s0
            if len(cur_rows) + glen > 128:
                tiles.append((cur_rows, cur_seg, cur_dst))
                cur_rows, cur_seg, cur_dst = [], [], []
                nseg = 0
            cur_rows.extend(tmpl_pos[s0:e0].tolist())
            cur_seg.extend([nseg] * glen)
            cur_dst.append(int(u))
            nseg += 1
        if cur_rows:
            tiles.append((cur_rows, cur_seg, cur_dst))
        walks.append(tiles)
        ntiles_max = max(ntiles_max, len(tiles))
    EPL = ntiles_max * 128

    for c in range(NCORES):
        tiles = walks[c]
        perm = np.zeros(EPL, dtype=np.int64)
        segid = np.zeros(EPL, dtype=np.int64)
        scat = np.full(EPL, TRASH, dtype=np.int64)
        for t in range(ntiles_max):
            base = t * 128
            if t < len(tiles):
                rows, sids, dsts = tiles[t]
                k = len(rows)
                ns = len(dsts)
                perm[base:base + k] = rows
                segid[base:base + k] = sids
                # pad positions in this tile -> slot ns (trash slot)
                segid[base + k:base + 128] = min(ns, 127)
                scat[base:base + ns] = dsts
            else:
                segid[base:base + 128] = 0
        per_core[c].update(perm=perm, segid=segid, scat=scat)

    # alpha gathers in dst-sorted order
    for c in range(NCORES):
        eids_sorted, slot_of_edge, dloc, dorder = dst_lists[c]
        es = eids_sorted[dorder]
        gsrc_d = np.zeros(EPL, dtype=np.int64)
        gdst_d = np.zeros(EPL, dtype=np.int64)
        get_d = np.zeros(EPL, dtype=np.int64)
        # scatter into tile walk order: position of edge k of the walk
        pos = []
        tiles = walks[c]
        for t, (rows, sids, dsts) in enumerate(tiles):
            pos.extend(range(t * 128, t * 128 + len(rows)))
        pos = np.asarray(pos, dtype=np.int64)
        gsrc_d[pos] = src[es]
        gdst_d[pos] = dst[es]
        get_d[pos] = et[es]
        # perm rows for pad positions point at row 0 (garbage, lands in trash)
        pc = per_core[c]
        pc.update(gsrc_d=gsrc_d, gdst_d=gdst_d, get_d=get_d)

    return dict(EP=EP, EPL=EPL, RC=RC, segs=segs, per_core=per_core,
                ntiles=ntiles_max)


# ----------------------------------------------------------------------------
# device program
# ----------------------------------------------------------------------------

def _build_program(sched):
    from concourse import bacc, mybir, tile

    f32 = mybir.dt.float32
    b16 = mybir.dt.bfloat16
    i16 = mybir.dt.int16

    EP, EPL, RC = sched["EP"], sched["EPL"], sched["RC"]
    NT = EPL // 128                       # dst-sorted tiles
    segs = sched["segs"]

    nc = bacc.Bacc("TRN2", target_bir_lowering=False, debug=False,
                   num_devices=NCORES)

    # ---- parameters ----
    x0 = nc.declare_dram_parameter("x0", [N_NODES, D], b16, isOutput=False)
    wtab = [nc.declare_dram_parameter(f"w{l}", [R, D], b16, isOutput=False)
            for l in (1, 2)]
    basisT = [nc.declare_dram_parameter(f"basisT{l}", [B, D * D], b16,
                                        isOutput=False) for l in (1, 2)]
    attT = [nc.declare_dram_parameter(f"attT{l}", [B, R], b16, isOutput=False)
            for l in (1, 2)]
    root = [nc.declare_dram_parameter(f"root{l}", [D, D], b16, isOutput=False)
            for l in (1, 2)]
    bias128 = [nc.declare_dram_parameter(f"bias{l}", [D, D], f32,
                                         isOutput=False) for l in (1, 2)]
    gi_t = nc.declare_dram_parameter("gi_t", [128, EP // 16], i16, isOutput=False)
    gi_src = nc.declare_dram_parameter("gi_src", [128, EPL // 16], i16, isOutput=False)
    gi_dst = nc.declare_dram_parameter("gi_dst", [128, EPL // 16], i16, isOutput=False)
    gi_et = nc.declare_dram_parameter("gi_et", [128, EPL // 16], i16, isOutput=False)
    gi_perm = nc.declare_dram_parameter("gi_perm", [128, EPL // 16], i16, isOutput=False)
    gi_own = nc.declare_dram_parameter("gi_own", [128, OWN_PAD // 16], i16, isOutput=False)
    gi_id = nc.declare_dram_parameter("gi_id", [128, 8], i16, isOutput=False)
    gi_scat = nc.declare_dram_parameter("gi_scat", [128, EPL // 16], i16, isOutput=False)
    segid_in = nc.declare_dram_parameter("segid", [128, NT], f32, isOutput=False)
    out_ext = nc.declare_dram_parameter("out", [NPC, D], f32, isOutput=True)

    # ---- internal DRAM ----
    msgT_hbm = nc.dram_tensor("msgT_hbm", [D, EP], b16)
    msg_rows = nc.dram_tensor("msg_rows", [EP, D], b16)
    alpha_hbm = nc.dram_tensor("alpha_hbm", [1, EPL], f32)
    hbuf = [nc.dram_tensor(f"hbuf{l}", [OWN_PAD, D], b16) for l in (1, 2)]
    h_own = nc.dram_tensor("h_own", [NPC, D], b16)
    h_full = nc.dram_tensor("h_full", [N_NODES, D], b16, addr_space="Shared")

    with tile.TileContext(nc) as tc:
        with tc.tile_pool(name="sb", bufs=1) as sb, \
             tc.tile_pool(name="st", bufs=3) as st, \
             tc.tile_pool(name="ps", bufs=5, space="PSUM") as ps, \
             tc.tile_pool(name="psd", bufs=1, space="PSUM") as psd:

            # ---------- constants / index tiles (loaded once) ----------
            def idx_tile(param, n16, tag):
                t = sb.tile([128, n16], i16, tag=tag)
                nc.sync.dma_start(out=t[:], in_=param[:])
                return t

            ti_t = idx_tile(gi_t, EP // 16, "ti_t")
            ti_src = idx_tile(gi_src, EPL // 16, "ti_src")
            ti_dst = idx_tile(gi_dst, EPL // 16, "ti_dst")
            ti_et = idx_tile(gi_et, EPL // 16, "ti_et")
            ti_perm = idx_tile(gi_perm, EPL // 16, "ti_perm")
            ti_own = idx_tile(gi_own, OWN_PAD // 16, "ti_own")
            ti_id = idx_tile(gi_id, 8, "ti_id")
            ti_scat = idx_tile(gi_scat, EPL // 16, "ti_scat")
            segid_sb = sb.tile([128, NT], f32, tag="segid")
            nc.sync.dma_start(out=segid_sb[:], in_=segid_in[:])
            iota = sb.tile([128, 128], f32, tag="iota")
            nc.gpsimd.iota(iota[:], pattern=[[1, 128]], base=0,
                           channel_multiplier=0,
                           allow_small_or_imprecise_dtypes=True)
            onecol = sb.tile([128, 1], b16, tag="onecol")
            nc.vector.memset(onecol[:], 1.0)
            zero20 = sb.tile([128, OWN_PAD // 128, D], b16, tag="zero20")
            nc.vector.memset(zero20[:], 0.0)

            for li, l in enumerate((1, 2)):
                xtab = x0 if l == 1 else h_full

                # zero the scatter accumulator
                nc.sync.dma_start(
                    out=hbuf[li].rearrange("(t p) o -> p t o", p=128),
                    in_=zero20[:])

                # ---------- gathers ----------
                xjT_t = sb.tile([128, EP], b16, tag="xjT_t")
                nc.gpsimd.dma_gather(xjT_t.rearrange("p (a e) -> p a e", a=1),
                                     xtab[:], ti_t[:], num_idxs=EP,
                                     num_idxs_reg=EP, elem_size=D,
                                     transpose=True)
                xjT_d = sb.tile([128, EPL], b16, tag="xjT_d")
                nc.gpsimd.dma_gather(xjT_d.rearrange("p (a e) -> p a e", a=1),
                                     xtab[:], ti_src[:], num_idxs=EPL,
                                     num_idxs_reg=EPL, elem_size=D,
                                     transpose=True)
                xiT_d = sb.tile([128, EPL], b16, tag="xiT_d")
                nc.gpsimd.dma_gather(xiT_d.rearrange("p (a e) -> p a e", a=1),
                                     xtab[:], ti_dst[:], num_idxs=EPL,
                                     num_idxs_reg=EPL, elem_size=D,
                                     transpose=True)
                wT_d = sb.tile([128, EPL], b16, tag="wT_d")
                nc.gpsimd.dma_gather(wT_d.rearrange("p (a e) -> p a e", a=1),
                                     wtab[li][:], ti_et[:], num_idxs=EPL,
                                     num_idxs_reg=EPL, elem_size=D,
                                     transpose=True)

                # ---------- alpha ----------
                zz = sb.tile([128, EPL], b16, tag="zz")
                nc.vector.tensor_mul(zz[:], xiT_d[:], wT_d[:])
                nc.vector.tensor_mul(zz[:], zz[:], xjT_d[:])
                for k in range(0, EPL, 512):
                    kw = min(EPL, k + 512) - k
                    a_ps = ps.tile([1, 512], f32, tag="bank")
                    nc.tensor.matmul(a_ps[:, :kw], onecol[:], zz[:, k:k + kw],
                                     start=True, stop=True)
                    a_sb = st.tile([1, 512], f32, tag="a_sb")
                    nc.scalar.copy(a_sb[:, :kw], a_ps[:, :kw])
                    nc.sync.dma_start(out=alpha_hbm[:, k:k + kw],
                                      in_=a_sb[:, :kw])
                al = sb.tile([128, NT], f32, tag="al")
                nc.sync.dma_start(
                    out=al[:], in_=alpha_hbm.rearrange("a (t p) -> (a p) t", p=128))
                av = sb.tile([128, NT], f32, tag="av")
                nc.scalar.activation(av[:], al[:],
                                     mybir.ActivationFunctionType.Exp)

                # ---------- Sa one-hot (a-scaled) ----------
                Sa = sb.tile([128, NT, 128], b16, tag="Sa")
                for t in range(NT):
                    nc.vector.tensor_scalar(Sa[:, t, :], iota[:],
                                            segid_sb[:, t:t + 1],
                                            av[:, t:t + 1],
                                            mybir.AluOpType.is_equal,
                                            mybir.AluOpType.mult)

                # ---------- W chunks + GEMM + bounce-out ----------
                bT = sb.tile([B, D * D], b16, tag="bT")
                nc.sync.dma_start(out=bT[:], in_=basisT[li][:])
                aT = sb.tile([B, R], b16, tag="aT")
                nc.sync.dma_start(out=aT[:], in_=attT[li][:])

                def load_W(wc):
                    r0, r1 = wc * RC, min(R, (wc + 1) * RC)
                    rc = r1 - r0
                    W = sb.tile([128, D, RC], b16, tag="W")
                    for o in range(D):
                        w_ps = ps.tile([128, 512], f32, tag="bank")
                        nc.tensor.matmul(w_ps[:, :rc],
                                         bT[:, o * D:(o + 1) * D],
                                         aT[:, r0:r1], start=True, stop=True)
                        eng = nc.vector if o % 2 == 0 else nc.scalar
                        if eng is nc.vector:
                            eng.tensor_copy(W[:, o, :rc], w_ps[:, :rc])
                        else:
                            eng.copy(W[:, o, :rc], w_ps[:, :rc])
                    return W

                W = None
                wc_loaded = -1
                seg_i = 0
                while seg_i < len(segs):
                    k0 = (segs[seg_i][1] // 512) * 512
                    k1 = min(EP, k0 + 512)
                    m_ps = ps.tile([128, 512], f32, tag="bank")
                    while seg_i < len(segs) and segs[seg_i][1] < k1:
                        r, lo, hi = segs[seg_i]
                        wc = r // RC
                        if wc != wc_loaded:
                            W = load_W(wc)
                            wc_loaded = wc
                        nc.tensor.matmul(
                            m_ps[:, lo - k0:hi - k0],
                            W[:, :, r - wc * RC],
                            xjT_t[:, lo:hi], start=True, stop=True)
                        seg_i += 1
                    m_sb = st.tile([128, 512], b16, tag="m_sb")
                    nc.vector.tensor_copy(m_sb[:, :k1 - k0],
                                          m_ps[:, :k1 - k0])
                    nc.sync.dma_start(out=msgT_hbm[:, k0:k1],
                                      in_=m_sb[:, :k1 - k0])
                    # transpose-gather this chunk to msg rows
                    mpm = st.tile([128, (k1 - k0) // 128, 128], b16,
                                  tag="mpm")
                    nc.gpsimd.dma_gather(
                        mpm[:], msgT_hbm[:, k0:k1], ti_id[:],
                        num_idxs=128, num_idxs_reg=128,
                        elem_size=k1 - k0, elem_step=EP, transpose=True)
                    nc.sync.dma_start(
                        out=msg_rows.rearrange("(c p) o -> p c o", p=128)
                            [:, k0 // 128:k1 // 128, :],
                        in_=mpm[:])

                # ---------- permutation gather + segment matmuls ----------
                msgd = sb.tile([128, NT, 128], b16, tag="msgd")
                nc.gpsimd.dma_gather(msgd[:], msg_rows[:], ti_perm[:],
                                     num_idxs=EPL, num_idxs_reg=EPL,
                                     elem_size=D)
                den_ps = psd.tile([128, NT], f32, tag="den")
                nm = sb.tile([128, NT, 128], b16, tag="nm")
                for t in range(NT):
                    s_ps = ps.tile([128, 512], f32, tag="bank")
                    nc.tensor.matmul(s_ps[:, :128], Sa[:, t, :], msgd[:, t, :],
                                     start=True, stop=True)
                    nc.tensor.matmul(den_ps[:, t:t + 1], Sa[:, t, :],
                                     onecol[:], start=True, stop=True)
                    rden = st.tile([128, 1], f32, tag="rden")
                    nc.vector.tensor_scalar_max(rden[:], den_ps[:, t:t + 1],
                                                1e-30)
                    nc.vector.reciprocal(rden[:], rden[:])
                    nc.vector.tensor_scalar_mul(nm[:, t, :], s_ps[:, :128],
                                                rden[:])
                nc.gpsimd.dma_scatter_add(hbuf[li][:], nm[:], ti_scat[:],
                                          num_idxs=EPL, num_idxs_reg=EPL,
                                          elem_size=D)

                # ---------- root term + bias + (relu) ----------
                xoT = sb.tile([128, OWN_PAD], b16, tag="xoT")
                nc.gpsimd.dma_gather(xoT.rearrange("p (a e) -> p a e", a=1),
                                     xtab[:], ti_own[:], num_idxs=OWN_PAD,
                                     num_idxs_reg=OWN_PAD, elem_size=D,
                                     transpose=True)
                rt = sb.tile([128, 128], b16, tag="rt")
                nc.sync.dma_start(out=rt[:], in_=root[li][:])
                bi = sb.tile([128, 128], f32, tag="bi")
                nc.sync.dma_start(out=bi[:], in_=bias128[li][:])
                hb = sb.tile([128, OWN_PAD // 128, D], b16, tag="hb")
                nc.sync.dma_start(
                    out=hb[:], in_=hbuf[li].rearrange("(t p) o -> p t o", p=128))
                res = sb.tile([128, OWN_PAD // 128, D],
                              b16 if l == 1 else f32, tag="res")
                for t in range(OWN_PAD // 128):
                    r_ps = ps.tile([128, 512], f32, tag="bank")
                    nc.tensor.matmul(r_ps[:, :128],
                                     xoT[:, t * 128:(t + 1) * 128],
                                     rt[:], start=True, stop=True)
                    tmp = st.tile([128, 128], f32, tag="tmp")
                    nc.vector.tensor_add(tmp[:], r_ps[:, :128], bi[:])
                    nc.vector.tensor_add(tmp[:], tmp[:], hb[:, t, :])
                    if l == 1:
                        nc.scalar.activation(res[:, t, :], tmp[:],
                                             mybir.ActivationFunctionType.Relu)
                    else:
                        nc.scalar.copy(res[:, t, :], tmp[:])
                if l == 1:
                    # store h rows 0..NPC, allgather
                    full_t = NPC // 128
                    rem = NPC - full_t * 128
                    nc.sync.dma_start(
                        out=h_own[:full_t * 128, :].rearrange(
                            "(t p) o -> p t o", p=128),
                        in_=res[:, :full_t, :])
                    if rem:
                        nc.sync.dma_start(
                            out=h_own[full_t * 128:NPC, :],
                            in_=res[:rem, full_t, :])
                    nc.gpsimd.collective_compute(
                        "AllGather", mybir.AluOpType.bypass,
                        replica_groups=[list(range(NCORES))],
                        ins=[h_own[:]], outs=[h_full[:]])
                else:
                    full_t = NPC // 128
                    rem = NPC - full_t * 128
                    nc.sync.dma_start(
                        out=out_ext[:full_t * 128, :].rearrange(
                            "(t p) o -> p t o", p=128),
                        in_=res[:, :full_t, :])
                    if rem:
                        nc.sync.dma_start(
                            out=out_ext[full_t * 128:NPC, :],
                            in_=res[:rem, full_t, :])

    nc.compile()
    return nc


# ----------------------------------------------------------------------------
# entry point
# ----------------------------------------------------------------------------

def kernel(entity, edge_index, edge_type, emb_table,
           basis1, att1, weight1, root1, bias1,
           basis2, att2, weight2, root2, bias2):
    entity = np.asarray(entity).astype(np.int64)
    edge_index = np.asarray(edge_index).astype(np.int64)
    et = np.asarray(edge_type).astype(np.int64)
    src, dst = edge_index[0], edge_index[1]

    x0 = np.asarray(emb_table, np.float32)[entity]
    sched = _build_schedule(src, dst, et)
    nc = _build_program(sched)

    def prep(basis, att, weight, root_, bias):
        return (np.ascontiguousarray(
                    np.asarray(basis, np.float32).transpose(0, 2, 1)
                    .reshape(B, D * D)).astype(bf16),
                np.ascontiguousarray(np.asarray(att, np.float32).T).astype(bf16),
                np.asarray(weight, np.float32).astype(bf16),
                np.asarray(root_, np.float32).astype(bf16),
                np.broadcast_to(np.asarray(bias, np.float32), (D, D)).copy())

    bT1, aT1, w1, rt1, bi1 = prep(basis1, att1, weight1, root1, bias1)
    bT2, aT2, w2, rt2, bi2 = prep(basis2, att2, weight2, root2, bias2)
    x0_b = x0.astype(bf16)

    EPL = sched["EPL"]
    NT = EPL // 128
    in_maps = []
    for c in range(NCORES):
        pc = sched["per_core"][c]
        segid_f = np.zeros((128, NT), np.float32)
        seg = pc["segid"].reshape(NT, 128).T        # [p, t]
        segid_f[:, :] = seg
        m = dict(x0=x0_b, w1=w1, w2=w2, basisT1=bT1, basisT2=bT2,
                 attT1=aT1, attT2=aT2, root1=rt1, root2=rt2,
                 bias1=bi1, bias2=bi2,
                 gi_t=_wrap_idx(pc["gsrc_t"]),
                 gi_src=_wrap_idx(pc["gsrc_d"]),
                 gi_dst=_wrap_idx(pc["gdst_d"]),
                 gi_et=_wrap_idx(pc["get_d"]),
                 gi_perm=_wrap_idx(pc["perm"]),
                 gi_own=_wrap_idx(np.minimum(np.arange(OWN_PAD), NPC - 1)
                                  + c * NPC),
                 gi_id=_wrap_idx(np.arange(128)),
                 gi_scat=_wrap_idx(pc["scat"]),
                 segid=segid_f)
        in_maps.append(m)

    if os.environ.get("BASS_GNN_SIM") == "1":
        from concourse.bass_interp import MultiCoreSim
        sim = MultiCoreSim(nc, num_cores=NCORES, require_finite=False,
                           require_nnan=False)
        for c in range(NCORES):
            for k, v in in_maps[c].items():
                sim.cores[c].tensor(k)[:] = v
        sim.simulate(check_with_hw=False)
        outs = [np.asarray(sim.cores[c].tensor("out")) for c in range(NCORES)]
    else:
        from concourse.bass_utils import run_bass_kernel_spmd
        trace = os.environ.get("BASS_GNN_TRACE") == "1"
        res = run_bass_kernel_spmd(nc, in_maps, core_ids=list(range(NCORES)),
                                   trace=trace)
        kernel.last_results = res
        outs = [res.results[c]["out"] for c in range(NCORES)]

    out = np.concatenate(outs, axis=0).astype(np.float32)
    return out
